# revision 37
# baseline (speedup 1.0000x reference)
"""Causal GQA self-attention (B=2,S=2048,H=2048,NH=16,NKV=4,HD=128) on 8 TRN2 cores.

Sharding: 2-way data-parallel over batch x 4-way tensor-parallel over heads.
Core c = 4*b + t handles batch b, q-heads 4t..4t+3, kv-head t (GQA group t).

Per-core pipeline (bf16 matmul operands, fp32 PSUM accumulate):
  - K/V projected in natural [keys, HD] layout (k-norm + rope cheap there),
    normalized k transposed to [HD, keys] via PE transpose.
  - Q projected directly transposed [HD, seq]; rms-norm via ones-matmul
    partition reduction; rotate-half via a signed permutation matmul.
  - scores computed transposed [keys, queries]; causal masking of the
    diagonal 128x128 blocks via a DVE 0/1 upper-triangular-mask multiply on
    the probs right after the exp (exact zeros, no extra PE work); softmax
    without max-subtraction (rms-normed q,k bound |score| <= sqrt(128)).
  - PV is INVERTED: probs (pt) are the stationary operand, v the moving one,
    giving y in natural [query, HD] layout; the softmax denominator comes
    almost free as a 1-column matmul against a ones vector reusing the
    already-loaded pt weights.
  - PSUM accumulators are memset-zeroed and accumulated with start=False so
    independent per-query-block groups can share a PSUM bank without
    tripping the bank-granular pending-zero semantics.
  - the AllToAll output is consumed with a LAG of TWO iterations (o_proj of
    iteration i runs inside iteration i+2) so each collective gets a full
    iteration to complete off the critical path; a scheduling-only dep pins
    the yta load late in the SP DMA queue (the ASAP scheduler would
    otherwise hoist it a full iteration early, where its collective-wait
    head-blocks every later load in the in-order queue).
  - DMA op count is minimized (each DMA costs ~1.8us of issue on HW): one x
    load per 512-query block, merged rope tables, y staged per-block as
    [dest-parity][head] so ONE 3-dim-AP store covers all 4 heads, one yta
    load, and bf16 output stores (host upcasts to f32).

`unroll` builds N chained copies of the full pipeline in one NEFF (used by
test.py to measure per-iteration HW time as a wall-clock slope).
"""
import sys
import os
import contextlib

for _p in ("/opt/trn_rl_repo", "/root/.axon_site/_ro/trn_rl_repo"):
    if os.path.isdir(_p) and _p not in sys.path:
        sys.path.insert(0, _p)

import numpy as np
import ml_dtypes
import concourse.bass as bass
import concourse.tile as tile
from concourse.tile import add_dep_helper
from concourse import bacc, mybir
from concourse.bass_utils import run_bass_kernel_spmd

B, S, H = 2, 2048, 2048
NH, NKV, HD = 16, 4, 128
EPS = 1e-6
P = 128
F32 = mybir.dt.float32
BF16 = mybir.dt.bfloat16
AF = mybir.ActivationFunctionType
ALU = mybir.AluOpType

_NC_CACHE = {}


def _oproj_quarter_half(nc, d, wo_t, yta, osp, psO, quarter, half, state):
    """Half of one (bp, qt) quarter of the output projection (32 matmuls);
    the second half also stores. Split so the halves can be emitted at the
    two attention-wave boundaries, covering the finalize latency with
    independent PE work."""
    bp, qt = quarter // 2, quarter % 2
    if half == 0:
        state["o_ps"] = [psO.tile([P, 512], F32, tag="oacc",
                                  name=f"ops_{bp}_{qt}_{oc}")
                         for oc in range(4)]
    o_ps = state["o_ps"]
    for hc in range(8 * half, 8 * half + 8):
        for oc in range(4):
            nc.tensor.matmul(
                o_ps[oc][:],
                yta[:, 16 * bp + hc, 128 * qt:128 * (qt + 1)],
                wo_t[:, hc, 512 * oc:512 * (oc + 1)],
                start=(hc == 0), stop=(hc == 15))
    if half == 1:
        osb4 = osp.tile([P, 4, 512], BF16, tag="osb")
        for oc in range(4):
            nc.vector.tensor_copy(osb4[:, oc, :], o_ps[oc][:])
        nc.sync.dma_start(
            d["o_out"].ap()[bp, 128 * qt:128 * (qt + 1), :], osb4[:])


def _emit_iteration(nc, tc, d, a2a_in, a2a_out, skip_collective, C, prev_src,
                    anchor=None):
    (wkv_t, wq_t, mrot_t, ones_t, ident_t, tri_t, ones1_t, epsk_t, epsq_t,
     wo_t, deferred) = C
    my_anchor = [None]
    with contextlib.ExitStack() as _st:
        _p = lambda *a, **k: _st.enter_context(tc.tile_pool(*a, **k))
        cp = _p(name="consts", bufs=1)
        sp = _p(name="stream", bufs=2)
        q2p = _p(name="q2p", bufs=2)
        t1p = _p(name="t1p", bufs=2)
        t2p = _p(name="t2p", bufs=2)
        q12p = _p(name="q12p", bufs=2)
        qsp = _p(name="qsb", bufs=2)
        sqp = _p(name="sqb", bufs=3)
        qnp = _p(name="qtn", bufs=5)
        ptp = _p(name="pt", bufs=8)
        yhp = _p(name="yhp", bufs=2)
        rdp = _p(name="rdp", bufs=4)
        kp = _p(name="ktmp", bufs=3)
        ytp = _p(name="ytaP", bufs=1)
        osp = _p(name="osbP", bufs=1)
        psBig = _p(name="psBig", bufs=3, space="PSUM")
        psY = _p(name="psY", bufs=2, space="PSUM")
        psO = _p(name="psO", bufs=1, space="PSUM")
        kT_all = cp.tile([P, 16, HD], BF16, tag="kT")
        v_all = cp.tile([P, 16, HD], BF16, tag="v")
        xh0 = sp.tile([P, 16, 512], BF16, tag="stream")
        nc.sync.dma_start(
            xh0[:], d["xT"].ap()[:, 0:512].rearrange("(c p) s -> p c s", p=P))
        if deferred is not None:
            deferred()
        yta = None
        if prev_src is not None:
            yta = ytp.tile([P, 32, 256], BF16, tag="yta")
            yta_dma = nc.sync.dma_start(
                yta[:], prev_src.rearrange("(c p) n -> p c n", p=P))
            if anchor is not None:
                # scheduling-only edge: without it the ASAP scheduler hoists
                # this load (whose only data dep is the lag-2 collective) a
                # full iteration early in the in-order SP DMA queue, where
                # its semaphore wait head-blocks every later load.
                add_dep_helper(yta_dma.ins, anchor.ins, sync=False,
                               reason="anti-hoist yta behind prev iteration")

        def load_tables(jq):
            csk_t = sp.tile([P, 4, 256], BF16, tag="csk",
                            name=f"csk_{jq}")
            nc.sync.dma_start(
                csk_t[:], d["csk"].ap()[512 * jq:512 * (jq + 1), :]
                .rearrange("(c p) n -> p c n", p=P))
            csq_t = sp.tile([P, 2, 512], BF16, tag="csq",
                            name=f"csq_{jq}")
            nc.sync.dma_start(csq_t[:],
                              d["csq"].ap()[:, :, 512 * jq:512 * (jq + 1)])
            return csk_t, csq_t

        tables0 = load_tables(0)

        # ---- main pass over 512-column blocks ----
        for jq in range(4):
            if jq == 0:
                xh, (csk_t, csq_t) = xh0, tables0
            else:
                xh, csk_t, csq_t = xh_next, csk_next, csq_next

            def xblk(hc):
                return xh[:, hc, :]

            cosq_t = csq_t[:, 0, :]
            sinq_t = csq_t[:, 1, :]

            # -- KV projection + k norm/rope for key tiles 4jq..4jq+3 --
            def emit_kv_pair(rp):
                kvp = psBig.tile([P, 2, 256], F32, tag="big", name=f"kvp_{rp}")
                for rr in range(2):
                    r = 2 * rp + rr
                    for hc in range(16):
                        nc.tensor.matmul(kvp[:, rr, :],
                                         xblk(hc)[:, 128 * r:128 * (r + 1)],
                                         wkv_t[:, hc, :],
                                         start=(hc == 0), stop=(hc == 15))
                for rr in range(2):
                    r = 2 * rp + rr
                    kt_i = 4 * jq + r
                    ksb = kp.tile([P, HD], BF16, tag="ksb")
                    nc.vector.tensor_copy(ksb[:], kvp[:, rr, 0:HD])
                    nc.vector.tensor_copy(v_all[:, kt_i, :], kvp[:, rr, HD:256])
                    kscr = kp.tile([P, HD], BF16, tag="kscr")
                    ks2 = kp.tile([P, 1], F32, tag="ks2")
                    nc.vector.scalar_tensor_tensor(
                        out=kscr[:], in0=ksb[:], scalar=1.0,
                        in1=ksb[:], op0=ALU.mult, op1=ALU.mult,
                        accum_out=ks2[:])
                    lnk = kp.tile([P, 1], F32, tag="lnk")
                    nc.scalar.activation(lnk[:], ks2[:], AF.Ln,
                                         bias=epsk_t[:], scale=1.0 / HD)
                    rk = kp.tile([P, 1], F32, tag="rk")
                    nc.scalar.activation(rk[:], lnk[:], AF.Exp, scale=-0.5)
                    t1k = kp.tile([P, HD], BF16, tag="t1k")
                    nc.vector.tensor_tensor(out=t1k[:], in0=ksb[:],
                                            in1=csk_t[:, r, 0:HD], op=ALU.mult)
                    t2k = kp.tile([P, HD], BF16, tag="t2k")
                    wrap = bass.AP(ksb.tensor, ksb.offset + 64,
                                   [list(ksb.ap[0]), [-64, 2], [1, 64]])
                    nc.vector.tensor_tensor(
                        out=t2k[:].rearrange("p (a b) -> p a b", a=2),
                        in0=wrap,
                        in1=csk_t[:, r, HD:256].rearrange("p (a b) -> p a b",
                                                          a=2),
                        op=ALU.mult)
                    k12 = kp.tile([P, HD], BF16, tag="k12")
                    nc.vector.tensor_tensor(out=k12[:], in0=t1k[:], in1=t2k[:],
                                            op=ALU.add)
                    khat = kp.tile([P, HD], BF16, tag="khat")
                    nc.vector.tensor_scalar_mul(khat[:], k12[:], rk[:])
                    ktr = psBig.tile([P, HD], BF16, tag="big",
                                     name=f"ktr_{kt_i}")
                    nc.tensor.transpose(ktr[:], khat[:], ident_t[:])
                    nc.vector.tensor_copy(kT_all[:, kt_i, :], ktr[:])

            emit_kv_pair(0)
            emit_kv_pair(1)

            # prefetch next block's x columns + rope tables (hides the 2MB
            # x DMA under this block's q-proj + attention)
            if jq < 3:
                xh_next = sp.tile([P, 16, 512], BF16, tag="stream",
                                  name=f"xh_{jq + 1}")
                xh_dma = nc.sync.dma_start(
                    xh_next[:],
                    d["xT"].ap()[:, 512 * (jq + 1):512 * (jq + 2)]
                    .rearrange("(c p) s -> p c s", p=P))
                if jq == 2:
                    my_anchor[0] = xh_dma
                csk_next, csq_next = load_tables(jq + 1)

            # -- Q proj + norm + rope for all 4 heads --
            qT_n = {}
            for h in range(4):
                q_ps = psBig.tile([P, 512], F32, tag="big")
                for hc in range(16):
                    nc.tensor.matmul(q_ps[:], wq_t[:, hc, 128 * h:128 * (h + 1)],
                                     xblk(hc),
                                     start=(hc == 0), stop=(hc == 15))
                qsb = qsp.tile([P, 512], BF16, tag="qsb")
                nc.vector.tensor_copy(qsb[:], q_ps[:])
                q2 = q2p.tile([P, 512], BF16, tag="q2")
                nc.vector.tensor_tensor(out=q2[:], in0=qsb[:], in1=qsb[:],
                                        op=ALU.mult)
                ssum_ps = psBig.tile([P, 512], F32, tag="big")
                nc.tensor.matmul(ssum_ps[:], ones_t[:], q2[:],
                                 start=True, stop=True)
                lnB = sqp.tile([P, 512], F32, tag="sqb")
                nc.scalar.activation(lnB[:], ssum_ps[:], AF.Ln,
                                     bias=epsq_t[:], scale=1.0)
                rqB = sqp.tile([P, 512], BF16, tag="sqbb")
                nc.scalar.activation(rqB[:], lnB[:], AF.Exp, scale=-0.5)
                rot_ps = psBig.tile([P, 512], F32, tag="big")
                nc.tensor.matmul(rot_ps[:], mrot_t[:], qsb[:],
                                 start=True, stop=True)
                t1 = t1p.tile([P, 512], BF16, tag="t1")
                nc.gpsimd.tensor_tensor(
                    out=t1[:], in0=qsb[:],
                    in1=cosq_t, op=ALU.mult)
                t2 = t2p.tile([P, 512], BF16, tag="t2")
                nc.vector.tensor_tensor(
                    out=t2[:], in0=rot_ps[:],
                    in1=sinq_t, op=ALU.mult)
                q12 = q12p.tile([P, 512], BF16, tag="q12")
                nc.vector.tensor_tensor(out=q12[:], in0=t1[:], in1=t2[:],
                                        op=ALU.add)
                qt = qnp.tile([P, 512], BF16, tag="qtn")
                nc.vector.tensor_tensor(out=qt[:], in0=q12[:], in1=rqB[:],
                                        op=ALU.mult)
                qT_n[h] = qt

            # -- attention: two 2-head waves, pt stationary / v moving --
            nch = 4 * jq + 4
            oproj_state = {}
            # per-jq y staging laid out [dest-parity a][head h][256=(b,hd)] so
            # ONE store DMA per jq covers all 4 heads ((a,h) merges into one
            # 8-count dim, keeping the AP within the 3-dim DMA limit)
            ytrJ = yhp.tile([P, 2, 4, 256], BF16, tag="ytrJ", bufs=1)
            for wv in range(2):
                heads = (2 * wv, 2 * wv + 1)
                y4 = {}
                for h in heads:
                    y4[h] = psY.tile([P, 4, HD], F32, tag="y4",
                                     name=f"y4_{jq}_{h}")
                    nc.vector.memset(y4[h][:], 0.0)
                den8 = psY.tile([P, 8], F32, tag="den8", bufs=2)
                nc.vector.memset(den8[:], 0.0)

                pts = {}

                def emit_sx(h, ci):
                    r = ci - 4 * jq
                    off = 128 * r if r >= 0 else 0
                    s_ps = psBig.tile([P, 512], F32, tag="big")
                    nc.tensor.matmul(s_ps[:, off:512], kT_all[:, ci, :],
                                     qT_n[h][:, off:512],
                                     start=True, stop=True)
                    pt = ptp.tile([P, 512], BF16, tag="pt")
                    nc.scalar.activation(pt[:, off:512], s_ps[:, off:512],
                                         AF.Exp)
                    if r >= 0:
                        # exact causal mask of the diagonal block: multiply
                        # by the 0/1 upper-inclusive mask (tri_t) on DVE
                        nc.vector.tensor_tensor(
                            out=pt[:, off:off + 128],
                            in0=pt[:, off:off + 128],
                            in1=tri_t[:], op=ALU.mult)
                    pts[(h, ci)] = pt

                def emit_pv(hl, h, ci):
                    r = ci - 4 * jq
                    pt = pts[(h, ci)]
                    for sub in range(4):
                        if r > sub:
                            continue
                        pt_sub = pt[:, 128 * sub:128 * (sub + 1)]
                        nc.tensor.matmul(y4[h][:, sub, :], pt_sub,
                                         v_all[:, ci, :],
                                         start=False, stop=(ci == 4 * jq + sub),
                                         skip_group_check=True)
                        nc.tensor.matmul(den8[:, 4 * hl + sub:4 * hl + sub + 1],
                                         pt_sub, ones1_t[:],
                                         start=False, stop=(ci == 4 * jq + sub),
                                         skip_group_check=True)

                for ci in range(nch):
                    for h in heads:
                        emit_sx(h, ci)
                    if ci > 1:
                        for hl, h in enumerate(heads):
                            emit_pv(hl, h, ci - 2)
                for cl in (nch - 2, nch - 1):
                    for hl, h in enumerate(heads):
                        emit_pv(hl, h, cl)

                if yta is not None:
                    _oproj_quarter_half(nc, d, wo_t, yta, osp, psO, jq, wv,
                                        oproj_state)

                for hl, h in enumerate(heads):
                    rden4 = rdp.tile([P, 4], F32, tag="rden")
                    nc.vector.reciprocal(rden4[:],
                                         den8[:, 4 * hl:4 * hl + 4])
                    yh4 = yhp.tile([P, 4, HD], BF16, tag="yh")
                    for sub in range(4):
                        nc.vector.tensor_scalar_mul(yh4[:, sub, :],
                                                    y4[h][:, sub, :],
                                                    rden4[:, sub:sub + 1])
                    ytr4 = psY.tile([P, 4, HD], BF16, tag="den8", bufs=2,
                                    name=f"ytr4_{jq}_{h}")
                    for sub in range(4):
                        nc.tensor.transpose(ytr4[:, sub, :], yh4[:, sub, :],
                                            ident_t[:])
                    # sub = 2a + b: parity a goes to ytrJ dim 1
                    nc.vector.tensor_copy(
                        ytrJ[:, :, h, :].rearrange("p a (b n) -> p a b n",
                                                   b=2),
                        ytr4[:].rearrange("p (a b) n -> p a b n", a=2))

            dstJ = bass.AP(
                a2a_in.tensor,
                a2a_in.offset + (512 * 2 * jq) * 256,
                [[256, 128], [128 * 256, 8], [1, 256]])
            srcJ = bass.AP(
                ytrJ.tensor, ytrJ.offset,
                [list(ytrJ.ap[0]), [256, 8], [1, 256]])
            nc.sync.dma_start(dstJ, srcJ)

        # ---- redistribute: 8-core AllToAll ----
        if not skip_collective:
            nc.gpsimd.collective_compute(
                "AllToAll", ALU.bypass,
                replica_groups=[[0, 1, 2, 3, 4, 5, 6, 7]],
                ins=[a2a_in.opt()],
                outs=[a2a_out.opt()])

    return (a2a_in if skip_collective else a2a_out), my_anchor[0]


def _build_nc(unroll=1, skip_collective=False):
    nc = bacc.Bacc("TRN2", target_bir_lowering=False, debug=False, num_devices=8)

    d = {}
    for name, shape in [
        ("xT", [H, S]), ("wq", [H, 512]), ("wkv", [H, 256]),
        ("csq", [HD, 2, S]), ("csk", [S, 256]), ("mrot", [HD, HD]),
        ("onesm", [P, P]), ("ident", [P, P]), ("tri", [P, P]),
        ("wo", [H, H]),
    ]:
        d[name] = nc.dram_tensor(name, shape, BF16, kind="ExternalInput")
    d["o_out"] = nc.dram_tensor("o_out", [2, 256, H], BF16,
                                kind="ExternalOutput")

    with tile.TileContext(nc) as tc:
        with (
            tc.tile_pool(name="dram", bufs=1, space="DRAM") as dram,
            tc.tile_pool(name="gconsts", bufs=1) as gp,
        ):
            a2a_in0 = dram.tile([8 * 512, 256], BF16, tag="a2a_in0")
            a2a_out0 = dram.tile([8 * 512, 256], BF16, tag="a2a_out0")
            a2a_in1 = dram.tile([8 * 512, 256], BF16, tag="a2a_in1")
            a2a_out1 = dram.tile([8 * 512, 256], BF16, tag="a2a_out1")
            abufs = [(a2a_in0, a2a_out0), (a2a_in1, a2a_out1)]
            wkv_t = gp.tile([P, 16, 256], BF16, tag="wkv")
            nc.sync.dma_start(
                wkv_t[:],
                d["wkv"].ap().rearrange("(c p) n -> p c n", p=P))
            wq_t = gp.tile([P, 16, 512], BF16, tag="wq")
            wo_t = gp.tile([P, 16, 2048], BF16, tag="wo")
            mrot_t = gp.tile([P, P], BF16, tag="mrot")
            nc.sync.dma_start(mrot_t[:], d["mrot"].ap())
            ones_t = gp.tile([P, P], BF16, tag="ones")
            nc.sync.dma_start(ones_t[:], d["onesm"].ap())
            ident_t = gp.tile([P, P], BF16, tag="ident")
            nc.sync.dma_start(ident_t[:], d["ident"].ap())
            tri_t = gp.tile([P, P], BF16, tag="tri")
            nc.sync.dma_start(tri_t[:], d["tri"].ap())
            ones1_t = gp.tile([P, 1], BF16, tag="ones1")
            nc.vector.memset(ones1_t[:], 1.0)

            def _deferred():
                nc.sync.dma_start(
                    wq_t[:],
                    d["wq"].ap().rearrange("(c p) n -> p c n", p=P))
                for q4 in range(4):
                    nc.sync.dma_start(
                        wo_t[:, 4 * q4:4 * (q4 + 1), :],
                        d["wo"].ap()[512 * q4:512 * (q4 + 1), :]
                        .rearrange("(c p) n -> p c n", p=P))
            epsk_t = gp.tile([P, 1], F32, tag="epsk")
            nc.vector.memset(epsk_t[:], EPS)
            epsq_t = gp.tile([P, 1], F32, tag="epsq")
            nc.vector.memset(epsq_t[:], HD * EPS)
            # o_proj consumes the a2a output from TWO iterations back, so
            # each AllToAll gets a full iteration to complete off the
            # critical path instead of racing the first attention wave.
            prev1 = prev2 = prev_anchor = None
            for it in range(unroll):
                a2a_in, a2a_out = abufs[it % 2]
                C = (wkv_t, wq_t, mrot_t, ones_t, ident_t, tri_t, ones1_t,
                     epsk_t, epsq_t, wo_t, _deferred if it == 0 else None)
                newest, prev_anchor = _emit_iteration(
                    nc, tc, d, a2a_in, a2a_out, skip_collective, C, prev2,
                    anchor=prev_anchor)
                prev2, prev1 = prev1, newest
            # tail: the last two iterations' o_proj are still pending
            with (
                tc.tile_pool(name="ytaF", bufs=2) as ytpF,
                tc.tile_pool(name="osbF", bufs=2) as ospF,
                tc.tile_pool(name="psOF", bufs=4, space="PSUM") as psOF,
            ):
                for srcp in (prev2, prev1):
                    if srcp is None:
                        continue
                    ytaF = ytpF.tile([P, 32, 256], BF16, tag="yta")
                    nc.sync.dma_start(ytaF[:],
                                      srcp.rearrange("(c p) n -> p c n", p=P))
                    for quarter in range(4):
                        stF = {}
                        _oproj_quarter_half(nc, d, wo_t, ytaF, ospF, psOF,
                                            quarter, 0, stF)
                        _oproj_quarter_half(nc, d, wo_t, ytaF, ospF, psOF,
                                            quarter, 1, stF)

    # Force Exp and Ln onto the shared 'natural_log_exp_and_others' ACT
    # table set: hide exp/ln from every other set during the act-table pass
    # (strict subsets, so the chosen set always really contains the func).
    import concourse.bacc as _bacc_mod
    _orig_tables = _bacc_mod.get_activation_tables

    def _patched_tables(arch):
        t = dict(_orig_tables(arch))
        for name in t:
            if name != "natural_log_exp_and_others":
                t[name] = t[name] - {AF.Exp, AF.Ln}
        return t

    _bacc_mod.get_activation_tables = _patched_tables
    try:
        nc.compile()
    finally:
        _bacc_mod.get_activation_tables = _orig_tables
    return nc


def _host_prep(x, rotary_cos, rotary_sin, Wq, Wk, Wv, Wo, q_norm_w, k_norm_w):
    """Shard + re-lay-out inputs for the 8 cores. Pure marshalling + table
    baking (no reductions)."""
    bf = ml_dtypes.bfloat16
    x = np.asarray(x, dtype=np.float32)
    cos = np.asarray(rotary_cos, dtype=np.float32)
    sin = np.asarray(rotary_sin, dtype=np.float32)
    Wq = np.asarray(Wq, dtype=np.float32)
    Wk = np.asarray(Wk, dtype=np.float32)
    Wv = np.asarray(Wv, dtype=np.float32)
    Wo = np.ascontiguousarray(np.asarray(Wo, dtype=np.float32).astype(bf))
    qw = np.asarray(q_norm_w, dtype=np.float32)
    kw = np.asarray(k_norm_w, dtype=np.float32)

    rot_idx = (np.arange(HD) + 64) % HD
    cosq = (cos * qw[None, :]).T
    sinq = (sin * qw[rot_idx][None, :]).T
    csq = np.ascontiguousarray(
        np.stack([cosq, sinq], axis=1).astype(bf))       # [HD, 2, S]
    Rm = np.zeros((HD, HD), dtype=np.float32)
    for dd in range(64):
        Rm[dd, dd + 64] = -1.0
        Rm[dd + 64, dd] = 1.0
    mrot = np.ascontiguousarray(Rm.T.astype(bf))
    cosk = cos * kw[None, :]
    sink = np.concatenate(
        [-sin[:, :64] * kw[None, 64:], sin[:, 64:] * kw[None, :64]], axis=1)
    csk = np.ascontiguousarray(
        np.concatenate([cosk, sink], axis=1).astype(bf))  # [S, 256]
    onesm = np.ones((P, P), dtype=bf)
    ident = np.eye(P, dtype=np.float32).astype(bf)
    # 0/1 mask: pt[k, q] valid iff k <= q within the diagonal block
    tri = np.ascontiguousarray(np.triu(np.ones((P, P), np.float32)).astype(bf))

    xT = [np.ascontiguousarray(x[b].T.astype(bf)) for b in range(B)]
    wq_s = [np.ascontiguousarray(Wq[:, t * 512:(t + 1) * 512].astype(bf))
            for t in range(4)]
    wkv_s = [np.ascontiguousarray(np.concatenate(
        [Wk[:, t * HD:(t + 1) * HD], Wv[:, t * HD:(t + 1) * HD]],
        axis=1).astype(bf)) for t in range(4)]

    in_maps = []
    for c in range(8):
        b, t = c // 4, c % 4
        in_maps.append({
            "xT": xT[b], "wq": wq_s[t], "wkv": wkv_s[t], "wo": Wo,
            "csq": csq, "csk": csk,
            "mrot": mrot, "onesm": onesm, "ident": ident, "tri": tri,
        })
    return in_maps


def kernel(**inputs):
    if "nc" not in _NC_CACHE:
        _NC_CACHE["nc"] = _build_nc()
    nc = _NC_CACHE["nc"]
    in_maps = _host_prep(**inputs)
    res = run_bass_kernel_spmd(nc, in_maps, list(range(8))).results
    out = np.empty((B, S, H), dtype=np.float32)
    for j in range(8):
        o = np.asarray(res[j]["o_out"], dtype=np.float32)
        for b in range(B):
            out[b, 256 * j:256 * (j + 1), :] = o[b]
    return out

